# revision 22
# baseline (speedup 1.0000x reference)
"""Trainium2 Bass kernel for nn_CapacitanceMatrix — v2 (transposed matmul).

C[b, i, j] = sigmoid(x[b]·Wd[i] + bd[i])        if i == j
           = -softplus(x[b]·Wo[m] + bo[m])      if i != j  (m = row-major off-diag idx)

v2 design (v1 baseline: 110.5us, x-stationary matmuls + f32 output):
- Transposed matmul: the 256 fused weight rows ([240 off-diag | 16 negated
  diag]) are the stationary lhsT in 128-row halves; x streams through as the
  512-wide moving operand. Output lands as [outs, batch] on PSUM partitions.
  No bias-seed matmuls: the bias rides the Exp activation's per-partition
  bias AP. 256 matmuls/core at 512-col streams vs 544 at 256.
- Epilogue per 1024-batch pair and 128-row half (psum = 2 banks):
    ev = exp(psum + b)       ScalarE, bf16 out
    off:  ot = -ln(1 + ev)   ScalarE Ln(bias=1) then DVE negate (bf16 2x)
    diag: ot = 1 / (1 + ev)  DVE add1 + reciprocal (psum holds -z)
- Output stays transposed in DRAM as [256, 8192] bf16 per core (2KB DMA
  rows); host gathers rows back to (i, j) order, casts f32, transposes.
- DMA: x fully SBUF-resident (16MB/core), 4KB input packets, separate rings
  for input (sync) / weights+bias (scalar) / output (gpsimd) so the output
  stream never head-of-line blocks input prefetch.
HBM traffic 21.3MB/core (16.8 in + 4.2 out) vs 25.8MB in v1.
"""

import sys

sys.path.insert(0, "/opt/trn_rl_repo")

from contextlib import ExitStack

import numpy as np

import concourse.bass as bass  # noqa: F401  (kept for parity with framework imports)
import concourse.tile as tile
from concourse import bacc, mybir
from concourse.bass_utils import run_bass_kernel_spmd

B = 65536
D = 1024
K = 16
NOUT = K * K  # 256
NCORES = 8
BC = B // NCORES  # 8192 rows per core
KD = D // 128  # 8 contraction chunks
BLKC = 2048  # batch columns per x block (4KB bf16 DMA rows)
NBLK = BC // BLKC  # 4
NPAIR = BC // 1024  # 8 epilogue pairs per core
NDIAG_P0 = 240 - 128  # partition where diag rows start in half B (112)

MM_DT_NAME = "bfloat16"  # kept for test.py compat

_CACHE = {}

_ACT_TABLES_PATCHED = False


def _pin_act_table_set():
    """Force Exp and Ln to resolve to the single LUT set that holds both
    (`natural_log_exp_and_others`) so the exp->ln alternation never thrashes
    ACT_TABLE_LOADs."""
    global _ACT_TABLES_PATCHED
    if _ACT_TABLES_PATCHED:
        return
    import concourse.hw_specs as hw_specs

    orig = hw_specs.get_activation_tables

    def patched(arch):
        tables = {k: set(v) for k, v in orig(arch).items()}
        keep = "natural_log_exp_and_others"
        if keep in tables:
            for k, v in tables.items():
                if k != keep:
                    v.discard(mybir.ActivationFunctionType.Exp)
                    v.discard(mybir.ActivationFunctionType.Ln)
        return tables

    bacc.get_activation_tables = patched
    _ACT_TABLES_PATCHED = True


def _build_bass():
    _pin_act_table_set()
    bf16 = mybir.dt.bfloat16
    f32 = mybir.dt.float32
    nc = bacc.Bacc("TRN2", target_bir_lowering=False, debug=False)
    # x pre-tiled on host: [block, chunk, partition, col]; every chunk DMA is
    # 128 fully contiguous 4KB partition rows
    xT = nc.dram_tensor("xT", [NBLK, KD, 128, BLKC], bf16, kind="ExternalInput").ap()
    # wts[p, c*256 + n] = W_dev[n, c*128 + p]; W_dev = [Wo; -Wd]
    wts = nc.dram_tensor("wts", [128, KD * NOUT], bf16, kind="ExternalInput").ap()
    # biasv[p, h] = bias for output row h*128+p ([bo; -bd])
    biasv = nc.dram_tensor("biasv", [128, 2], f32, kind="ExternalInput").ap()
    # transposed output: row r = fused weight row, col = batch index in core
    outT = nc.dram_tensor("outT", [NOUT, BC], bf16, kind="ExternalOutput").ap()

    with tile.TileContext(nc) as tc, ExitStack() as ctx:
        const_pool = ctx.enter_context(tc.tile_pool(name="const", bufs=1))
        # 2 blocks in flight: input runs ~1.5 blocks ahead of the PE at
        # steady state, and fewer pool buffers shorten the semaphore
        # init/teardown cascades
        x_pool = ctx.enter_context(tc.tile_pool(name="x", bufs=2 * KD))
        ev_pool = ctx.enter_context(tc.tile_pool(name="ev", bufs=4))
        sp_pool = ctx.enter_context(tc.tile_pool(name="sp", bufs=16))
        dt_pool = ctx.enter_context(tc.tile_pool(name="dt", bufs=8))
        psum_pool = ctx.enter_context(tc.tile_pool(name="ps", bufs=4, space="PSUM"))

        # weights staged per chunk-pair tile so the c0 matmuls aren't gated
        # on the full 0.5MB const load (tile deps are tile-granular)
        wt_tiles = []
        for c2 in range(KD // 2):
            wtc = const_pool.tile([128, 2 * NOUT], bf16, tag=f"wt{c2}")
            nc.scalar.dma_start(wtc[:], wts[:, c2 * 2 * NOUT : (c2 + 1) * 2 * NOUT])
            wt_tiles.append(wtc)
        bias_sb = const_pool.tile([128, 2], f32, tag="bias")
        nc.scalar.dma_start(bias_sb[:], biasv)

        # warm the PE's DVFS p-state with throwaway matmuls while the first
        # weights/x DMAs are in flight: after an idle period the PE runs at
        # ~half clock for its first ~3us of work, which would tax the real
        # stream's head
        dum_sb = const_pool.tile([128, 512], bf16, tag="dum")
        nc.vector.memset(dum_sb[:], 0.0)
        ps_dum = psum_pool.tile([128, 1024], f32, tag="ps")
        for _ in range(12):
            nc.tensor.matmul(
                ps_dum[0:16, 0:512],
                lhsT=dum_sb[:, 0:16],
                rhs=dum_sb[:],
                start=True,
                stop=True,
                skip_group_check=True,
            )

        def lhsT_of(c, half):
            return wt_tiles[c // 2][
                :, (c % 2) * NOUT + half * 128 : (c % 2) * NOUT + half * 128 + 128
            ]

        for blk in range(NBLK):
            if blk == 0:
                # stage block 0 in pair-sized tiles, issued pair-major so the
                # in-order input ring delivers everything pair 0 needs first
                xs = [[None] * (BLKC // 1024) for _ in range(KD)]
                for hp in range(BLKC // 1024):
                    for c in range(KD):
                        xp = x_pool.tile([128, 1024], bf16, tag="x0")
                        nc.sync.dma_start(
                            xp[:], xT[blk, c, :, hp * 1024 : (hp + 1) * 1024]
                        )
                        xs[c][hp] = xp
                xcol = lambda c, col0, a, b: xs[c][col0 // 1024][:, a:b]
            else:
                xf = []
                for c in range(KD):
                    xc = x_pool.tile([128, BLKC], bf16, tag="x")
                    nc.sync.dma_start(xc[:], xT[blk, c])
                    xf.append(xc)
                xcol = lambda c, col0, a, b: xf[c][:, col0 + a : col0 + b]
            for hp in range(BLKC // 1024):
                pp = blk * (BLKC // 1024) + hp
                col0 = hp * 1024
                last_pair = pp == NPAIR - 1
                # half B (3-deep ScalarE chain) first, half A (2-deep) last so
                # the drain after the final matmul is as short as possible
                for half in (1, 0):
                    ps = psum_pool.tile([128, 1024], f32, tag="ps")
                    for c in range(KD):
                        lhsT = lhsT_of(c, half)
                        for g in range(2):
                            nc.tensor.matmul(
                                ps[:, g * 512 : (g + 1) * 512],
                                lhsT=lhsT,
                                rhs=xcol(c, col0, g * 512, (g + 1) * 512),
                                start=(c == 0),
                                stop=(c == KD - 1),
                                skip_group_check=True,
                            )
                    # off-diag rows ship as +softplus (the host negates during
                    # the f32 gather), so the DVE never touches the data
                    cols = slice(pp * 1024, (pp + 1) * 1024)
                    ev = ev_pool.tile([128, 1024], bf16, tag="ev")
                    nc.scalar.activation(
                        ev[:],
                        ps[:],
                        mybir.ActivationFunctionType.Exp,
                        bias=bias_sb[:, half : half + 1],
                    )
                    sp = sp_pool.tile([128, 1024], bf16, tag="sp")
                    nc.scalar.activation(
                        sp[:], ev[:], mybir.ActivationFunctionType.Ln, bias=1.0
                    )
                    if half == 0:
                        nc.gpsimd.dma_start(outT[0:128, cols], sp[:])
                    else:
                        # diag rows sit at [112:128] where psum held -z, so
                        # sp = ln(1+e^-z) = softplus(-z) there and
                        # sigmoid(z) = exp(-sp). Compute-engine APs need
                        # 32-aligned partition starts, so the diag Exp runs
                        # on [96:128] into a scratch tile (96..111 junk never
                        # DMA'd out; DMA APs have no alignment limit).
                        p0 = NDIAG_P0  # 112
                        sc = dt_pool.tile([128, 1024], bf16, tag="sc")
                        nc.scalar.activation(
                            sc[96:128],
                            sp[96:128],
                            mybir.ActivationFunctionType.Exp,
                            scale=-1.0,
                        )
                        nc.gpsimd.dma_start(outT[128 : 128 + p0, cols], sp[0:p0])
                        nc.gpsimd.dma_start(
                            outT[128 + p0 : 256, cols], sc[p0:128]
                        )
    nc.compile()
    return nc


def _get_nc():
    if "nc" not in _CACHE:
        _CACHE["nc"] = _build_bass()
    return _CACHE["nc"]


def _host_prep(x, Wd, bd, Wo, bo):
    import ml_dtypes

    np_bf16 = ml_dtypes.bfloat16
    # fused rows: [Wo (240) ; -Wd (16)] — diag negated so psum holds -z and
    # sigmoid(z) = 1/(1 + e^-z) comes out of the shared exp pass
    w_dev = np.concatenate([Wo, -Wd], axis=0)  # (256, D)
    b_dev = np.concatenate([bo, -bd], axis=0)  # (256,)
    wts = np.ascontiguousarray(
        w_dev.T.reshape(KD, 128, NOUT).transpose(1, 0, 2).reshape(128, KD * NOUT)
    ).astype(np_bf16)
    biasv = np.ascontiguousarray(
        np.stack([b_dev[0:128], b_dev[128:256]], axis=1)
    ).astype(np.float32)
    in_maps = []
    for c in range(NCORES):
        xs = x[c * BC : (c + 1) * BC]  # (BC, D)
        # -> (NBLK, KD, 128, BLKC): elem (blk, kd, p, t) = xs[blk*BLKC+t, kd*128+p]
        xTc = np.ascontiguousarray(
            xs.reshape(NBLK, BLKC, KD, 128).transpose(0, 2, 3, 1)
        ).astype(np_bf16)
        in_maps.append({"xT": xTc, "wts": wts, "biasv": biasv})
    return in_maps


def _install_env_shims():
    """The agent image's `antenv` stub lacks `axon_hooks`; bass_utils imports
    it on any trace=True/BASS_TRACE run. Provide it (wired to the ctypes NTFF
    hook when available), and skip the S3 artifact upload (no egress)."""
    if "antenv.axon_hooks" in sys.modules:
        return
    import types

    try:
        import antenv
    except ImportError:
        return
    if hasattr(antenv, "axon_hooks"):
        return
    mod = types.ModuleType("antenv.axon_hooks")
    hook = [None]
    try:
        from trn_agent_boot.trn_boot import _ntff_profile_via_ctypes

        hook[0] = _ntff_profile_via_ctypes("/opt/axon/libaxon_pjrt.so")
    except Exception:
        pass
    mod.set_axon_ntff_profile_hook = lambda h: hook.__setitem__(0, h)
    mod.get_axon_ntff_profile_hook = lambda: hook[0]
    sys.modules["antenv.axon_hooks"] = mod
    antenv.axon_hooks = mod

    import concourse.bass_utils as bu

    bu.upload_artifacts = lambda tmpdir: tmpdir


def _run(in_maps, **kwargs):
    _install_env_shims()
    nc = _get_nc()
    return run_bass_kernel_spmd(nc, in_maps, list(range(NCORES)), **kwargs)


# row r of outT -> flat (i, j) position: P[i*16+j] = source row
def _out_perm():
    off_i, off_j = np.nonzero(~np.eye(K, dtype=bool))
    P = np.empty(NOUT, np.int64)
    P[off_i * K + off_j] = np.arange(K * (K - 1))
    P[np.arange(K) * (K + 1)] = K * (K - 1) + np.arange(K)
    return P


def kernel(x, Wd, bd, Wo, bo, _bench_results=None, **kwargs):
    x = np.asarray(x, np.float32)
    in_maps = _host_prep(
        x,
        np.asarray(Wd, np.float32),
        np.asarray(bd, np.float32),
        np.asarray(Wo, np.float32),
        np.asarray(bo, np.float32),
    )
    res = _run(in_maps, **kwargs)
    if _bench_results is not None:
        _bench_results.append(res)
    P = _out_perm()
    out = np.empty((B, NOUT), np.float32)
    for c in range(NCORES):
        oT = np.asarray(res.results[c]["outT"], dtype=np.float32)  # (256, BC)
        # device ships +softplus for the 240 off-diag rows; negate here
        oT[: K * (K - 1)] *= -1.0
        out[c * BC : (c + 1) * BC] = oT[P].T
    return out.reshape(B, K, K)


# revision 23
# speedup vs baseline: 1.0836x; 1.0836x over previous
"""Trainium2 Bass kernel for nn_CapacitanceMatrix — v2 (transposed matmul).

C[b, i, j] = sigmoid(x[b]·Wd[i] + bd[i])        if i == j
           = -softplus(x[b]·Wo[m] + bo[m])      if i != j  (m = row-major off-diag idx)

v2 design (v1 baseline: 110.5us, x-stationary matmuls + f32 output):
- Transposed matmul: the 256 fused weight rows ([240 off-diag | 16 negated
  diag]) are the stationary lhsT in 128-row halves; x streams through as the
  512-wide moving operand. Output lands as [outs, batch] on PSUM partitions.
  No bias-seed matmuls: the bias rides the Exp activation's per-partition
  bias AP. 256 matmuls/core at 512-col streams vs 544 at 256.
- Epilogue per 1024-batch pair and 128-row half (psum = 2 banks):
    ev = exp(psum + b)       ScalarE, bf16 out
    off:  ot = -ln(1 + ev)   ScalarE Ln(bias=1) then DVE negate (bf16 2x)
    diag: ot = 1 / (1 + ev)  DVE add1 + reciprocal (psum holds -z)
- Output stays transposed in DRAM as [256, 8192] bf16 per core (2KB DMA
  rows); host gathers rows back to (i, j) order, casts f32, transposes.
- DMA: x fully SBUF-resident (16MB/core), 4KB input packets, separate rings
  for input (sync) / weights+bias (scalar) / output (gpsimd) so the output
  stream never head-of-line blocks input prefetch.
HBM traffic 21.3MB/core (16.8 in + 4.2 out) vs 25.8MB in v1.
"""

import sys

sys.path.insert(0, "/opt/trn_rl_repo")

from contextlib import ExitStack

import numpy as np

import concourse.bass as bass  # noqa: F401  (kept for parity with framework imports)
import concourse.tile as tile
from concourse import bacc, mybir
from concourse.bass_utils import run_bass_kernel_spmd

B = 65536
D = 1024
K = 16
NOUT = K * K  # 256
NCORES = 8
BC = B // NCORES  # 8192 rows per core
KD = D // 128  # 8 contraction chunks
BLKC = 2048  # batch columns per x block (4KB bf16 DMA rows)
NBLK = BC // BLKC  # 4
NPAIR = BC // 1024  # 8 epilogue pairs per core
NDIAG_P0 = 240 - 128  # partition where diag rows start in half B (112)

MM_DT_NAME = "bfloat16"  # kept for test.py compat

_CACHE = {}

_ACT_TABLES_PATCHED = False


def _pin_act_table_set():
    """Force Exp and Ln to resolve to the single LUT set that holds both
    (`natural_log_exp_and_others`) so the exp->ln alternation never thrashes
    ACT_TABLE_LOADs."""
    global _ACT_TABLES_PATCHED
    if _ACT_TABLES_PATCHED:
        return
    import concourse.hw_specs as hw_specs

    orig = hw_specs.get_activation_tables

    def patched(arch):
        tables = {k: set(v) for k, v in orig(arch).items()}
        keep = "natural_log_exp_and_others"
        if keep in tables:
            for k, v in tables.items():
                if k != keep:
                    v.discard(mybir.ActivationFunctionType.Exp)
                    v.discard(mybir.ActivationFunctionType.Ln)
        return tables

    bacc.get_activation_tables = patched
    _ACT_TABLES_PATCHED = True


def _build_bass():
    _pin_act_table_set()
    bf16 = mybir.dt.bfloat16
    f32 = mybir.dt.float32
    nc = bacc.Bacc("TRN2", target_bir_lowering=False, debug=False)
    # x pre-tiled on host: [block, chunk, partition, col]; every chunk DMA is
    # 128 fully contiguous 4KB partition rows
    xT = nc.dram_tensor("xT", [NBLK, KD, 128, BLKC], bf16, kind="ExternalInput").ap()
    # wts[p, c*256 + n] = W_dev[n, c*128 + p]; W_dev = [Wo; -Wd]
    wts = nc.dram_tensor("wts", [128, KD * NOUT], bf16, kind="ExternalInput").ap()
    # biasv[p, h] = bias for output row h*128+p ([bo; -bd])
    biasv = nc.dram_tensor("biasv", [128, 2], f32, kind="ExternalInput").ap()
    # transposed output: row r = fused weight row, col = batch index in core
    outT = nc.dram_tensor("outT", [NOUT, BC], bf16, kind="ExternalOutput").ap()

    with tile.TileContext(nc) as tc, ExitStack() as ctx:
        const_pool = ctx.enter_context(tc.tile_pool(name="const", bufs=1))
        # 2 blocks in flight: input runs ~1.5 blocks ahead of the PE at
        # steady state, and fewer pool buffers shorten the semaphore
        # init/teardown cascades
        x_pool = ctx.enter_context(tc.tile_pool(name="x", bufs=4 * KD))
        ev_pool = ctx.enter_context(tc.tile_pool(name="ev", bufs=4))
        sp_pool = ctx.enter_context(tc.tile_pool(name="sp", bufs=16))
        dt_pool = ctx.enter_context(tc.tile_pool(name="dt", bufs=8))
        psum_pool = ctx.enter_context(tc.tile_pool(name="ps", bufs=4, space="PSUM"))

        # weights staged per chunk-pair tile so the c0 matmuls aren't gated
        # on the full 0.5MB const load (tile deps are tile-granular)
        wt_tiles = []
        for c2 in range(KD // 2):
            wtc = const_pool.tile([128, 2 * NOUT], bf16, tag=f"wt{c2}")
            nc.scalar.dma_start(wtc[:], wts[:, c2 * 2 * NOUT : (c2 + 1) * 2 * NOUT])
            wt_tiles.append(wtc)
        bias_sb = const_pool.tile([128, 2], f32, tag="bias")
        nc.scalar.dma_start(bias_sb[:], biasv)

        # warm the PE's DVFS p-state with throwaway matmuls while the first
        # weights/x DMAs are in flight: after an idle period the PE runs at
        # ~half clock for its first ~3us of work, which would tax the real
        # stream's head
        dum_sb = const_pool.tile([128, 512], bf16, tag="dum")
        nc.vector.memset(dum_sb[:], 0.0)
        ps_dum = psum_pool.tile([128, 1024], f32, tag="ps")
        for _ in range(10):
            nc.tensor.matmul(
                ps_dum[0:16, 0:512],
                lhsT=dum_sb[:, 0:16],
                rhs=dum_sb[:],
                start=True,
                stop=True,
                skip_group_check=True,
            )

        def lhsT_of(c, half):
            return wt_tiles[c // 2][
                :, (c % 2) * NOUT + half * 128 : (c % 2) * NOUT + half * 128 + 128
            ]

        for blk in range(NBLK):
            # all input staged in pair-sized tiles, issued pair-major, so the
            # in-order input ring always delivers a pair's chunks just ahead
            # of the PE consuming them — the PE must never stall (any idle
            # gap drops it to ~half clock for its next ~3us of work)
            xs = [[None] * (BLKC // 1024) for _ in range(KD)]
            for hp in range(BLKC // 1024):
                for c in range(KD):
                    xp = x_pool.tile([128, 1024], bf16, tag="x0")
                    nc.sync.dma_start(
                        xp[:], xT[blk, c, :, hp * 1024 : (hp + 1) * 1024]
                    )
                    xs[c][hp] = xp
            xcol = lambda c, col0, a, b: xs[c][col0 // 1024][:, a:b]
            for hp in range(BLKC // 1024):
                pp = blk * (BLKC // 1024) + hp
                col0 = hp * 1024
                last_pair = pp == NPAIR - 1
                # half B (3-deep ScalarE chain) first, half A (2-deep) last so
                # the drain after the final matmul is as short as possible
                for half in (1, 0):
                    ps = psum_pool.tile([128, 1024], f32, tag="ps")
                    for c in range(KD):
                        lhsT = lhsT_of(c, half)
                        for g in range(2):
                            nc.tensor.matmul(
                                ps[:, g * 512 : (g + 1) * 512],
                                lhsT=lhsT,
                                rhs=xcol(c, col0, g * 512, (g + 1) * 512),
                                start=(c == 0),
                                stop=(c == KD - 1),
                                skip_group_check=True,
                            )
                    # off-diag rows ship as +softplus (the host negates during
                    # the f32 gather), so the DVE never touches the data
                    cols = slice(pp * 1024, (pp + 1) * 1024)
                    ev = ev_pool.tile([128, 1024], bf16, tag="ev")
                    nc.scalar.activation(
                        ev[:],
                        ps[:],
                        mybir.ActivationFunctionType.Exp,
                        bias=bias_sb[:, half : half + 1],
                    )
                    sp = sp_pool.tile([128, 1024], bf16, tag="sp")
                    nc.scalar.activation(
                        sp[:], ev[:], mybir.ActivationFunctionType.Ln, bias=1.0
                    )
                    if half == 0:
                        nc.gpsimd.dma_start(outT[0:128, cols], sp[:])
                    else:
                        # diag rows sit at [112:128] where psum held -z, so
                        # sp = ln(1+e^-z) = softplus(-z) there and
                        # sigmoid(z) = exp(-sp). Compute-engine APs need
                        # 32-aligned partition starts, so the diag Exp runs
                        # on [96:128] into a scratch tile (96..111 junk never
                        # DMA'd out; DMA APs have no alignment limit).
                        p0 = NDIAG_P0  # 112
                        sc = dt_pool.tile([128, 1024], bf16, tag="sc")
                        nc.scalar.activation(
                            sc[96:128],
                            sp[96:128],
                            mybir.ActivationFunctionType.Exp,
                            scale=-1.0,
                        )
                        nc.gpsimd.dma_start(outT[128 : 128 + p0, cols], sp[0:p0])
                        nc.gpsimd.dma_start(
                            outT[128 + p0 : 256, cols], sc[p0:128]
                        )
    nc.compile()
    return nc


def _get_nc():
    if "nc" not in _CACHE:
        _CACHE["nc"] = _build_bass()
    return _CACHE["nc"]


def _host_prep(x, Wd, bd, Wo, bo):
    import ml_dtypes

    np_bf16 = ml_dtypes.bfloat16
    # fused rows: [Wo (240) ; -Wd (16)] — diag negated so psum holds -z and
    # sigmoid(z) = 1/(1 + e^-z) comes out of the shared exp pass
    w_dev = np.concatenate([Wo, -Wd], axis=0)  # (256, D)
    b_dev = np.concatenate([bo, -bd], axis=0)  # (256,)
    wts = np.ascontiguousarray(
        w_dev.T.reshape(KD, 128, NOUT).transpose(1, 0, 2).reshape(128, KD * NOUT)
    ).astype(np_bf16)
    biasv = np.ascontiguousarray(
        np.stack([b_dev[0:128], b_dev[128:256]], axis=1)
    ).astype(np.float32)
    in_maps = []
    for c in range(NCORES):
        xs = x[c * BC : (c + 1) * BC]  # (BC, D)
        # -> (NBLK, KD, 128, BLKC): elem (blk, kd, p, t) = xs[blk*BLKC+t, kd*128+p]
        xTc = np.ascontiguousarray(
            xs.reshape(NBLK, BLKC, KD, 128).transpose(0, 2, 3, 1)
        ).astype(np_bf16)
        in_maps.append({"xT": xTc, "wts": wts, "biasv": biasv})
    return in_maps


def _install_env_shims():
    """The agent image's `antenv` stub lacks `axon_hooks`; bass_utils imports
    it on any trace=True/BASS_TRACE run. Provide it (wired to the ctypes NTFF
    hook when available), and skip the S3 artifact upload (no egress)."""
    if "antenv.axon_hooks" in sys.modules:
        return
    import types

    try:
        import antenv
    except ImportError:
        return
    if hasattr(antenv, "axon_hooks"):
        return
    mod = types.ModuleType("antenv.axon_hooks")
    hook = [None]
    try:
        from trn_agent_boot.trn_boot import _ntff_profile_via_ctypes

        hook[0] = _ntff_profile_via_ctypes("/opt/axon/libaxon_pjrt.so")
    except Exception:
        pass
    mod.set_axon_ntff_profile_hook = lambda h: hook.__setitem__(0, h)
    mod.get_axon_ntff_profile_hook = lambda: hook[0]
    sys.modules["antenv.axon_hooks"] = mod
    antenv.axon_hooks = mod

    import concourse.bass_utils as bu

    bu.upload_artifacts = lambda tmpdir: tmpdir


def _run(in_maps, **kwargs):
    _install_env_shims()
    nc = _get_nc()
    return run_bass_kernel_spmd(nc, in_maps, list(range(NCORES)), **kwargs)


# row r of outT -> flat (i, j) position: P[i*16+j] = source row
def _out_perm():
    off_i, off_j = np.nonzero(~np.eye(K, dtype=bool))
    P = np.empty(NOUT, np.int64)
    P[off_i * K + off_j] = np.arange(K * (K - 1))
    P[np.arange(K) * (K + 1)] = K * (K - 1) + np.arange(K)
    return P


def kernel(x, Wd, bd, Wo, bo, _bench_results=None, **kwargs):
    x = np.asarray(x, np.float32)
    in_maps = _host_prep(
        x,
        np.asarray(Wd, np.float32),
        np.asarray(bd, np.float32),
        np.asarray(Wo, np.float32),
        np.asarray(bo, np.float32),
    )
    res = _run(in_maps, **kwargs)
    if _bench_results is not None:
        _bench_results.append(res)
    P = _out_perm()
    out = np.empty((B, NOUT), np.float32)
    for c in range(NCORES):
        oT = np.asarray(res.results[c]["outT"], dtype=np.float32)  # (256, BC)
        # device ships +softplus for the 240 off-diag rows; negate here
        oT[: K * (K - 1)] *= -1.0
        out[c * BC : (c + 1) * BC] = oT[P].T
    return out.reshape(B, K, K)
